# revision 1
# baseline (speedup 1.0000x reference)
"""DySample (dynamic upsampling) Trainium2 Bass kernel.

Math (per sample):
  off = tanh(pixel_shuffle(w @ x + b))            # per-pixel offsets, 8 ch -> (g, {x,y}, i, j)
  grid = static_base + off * 0.125                # normalized coords
  out  = bilinear_border_sample(x_grouped, grid)  # [4g, 64c, 192, 192]

Strategy: pure data-parallel over batch B=8 -> 1 sample per NeuronCore.
Per core:
  - PE computes the 1x1 conv (32x256 weight) into PSUM, pixel-major
    (psum [128 px, 32 och]); ACT applies tanh.
  - DVE computes per-pixel gather indices (int32) and 4 bilinear tap
    weights (fp32 fields, [128, 72] tiles; pixel p~ = p*72 + b).
  - SWDGE indirect DMA gathers, per output pixel, one 512B quad
    [TL|BL|TR|BR] (4 taps x 64 ch, fp16) from a host-staged row-pair-
    interleaved image in HBM.
  - DVE+GPSIMD apply the 4 per-partition tap weights (scalar_tensor_tensor
    chains) and write fp16 outputs; HWDGE DMA stores contiguous runs.
Host reassembles/reshapes (pure layout + dtype cast, no arithmetic).
"""

import os
from contextlib import ExitStack

import numpy as np

import concourse.bass as bass
import concourse.tile as tile
from concourse import bacc
from concourse import mybir
from concourse.bass import IndirectOffsetOnAxis
from concourse.bass_utils import run_bass_kernel_spmd

F16 = mybir.dt.float16
F32 = mybir.dt.float32
I32 = mybir.dt.int32
U32 = mybir.dt.uint32
A = mybir.AluOpType
AF = mybir.ActivationFunctionType

P = 128          # partitions
NPX = 9216       # 96*96 conv-resolution pixels per (group, parity) unit
NB = NPX // P    # 72 free-dim blocks per unit
HB = NB // 2     # half-unit blocks (gather chunk)
GROUPS = 4
UNITS = 16       # (g, i, j)
XG_ROWS = 97 * 96           # rows per group in the interleaved gather image
XG_ALLOC = XG_ROWS + 2      # +2 pad rows so the overlapping gather AP stays in bounds
XG_TOT = GROUPS * XG_ALLOC
NCORES = 8

# column j of the channel-major conv input holds flat pixel (j%128)*72 + j//128,
# so conv tile t / psum partition p <-> pixel p*72 + t  (partition-major raster)
_PERM = (np.arange(NPX) % P) * NB + (np.arange(NPX) // P)

_CACHE = {}


def _build_nc():
    nc = bacc.Bacc("TRN2", target_bir_lowering=False,
                   dynamic_dma_scratch_size=65536)
    x_cm = nc.dram_tensor("x_cm", [2, P, NPX], F16, kind="ExternalInput")
    wt = nc.dram_tensor("wt", [2, P, 32], F16, kind="ExternalInput")
    xg = nc.dram_tensor("xg", [XG_TOT, P], F16, kind="ExternalInput")
    bxd = nc.dram_tensor("bx", [2, P, NB], F32, kind="ExternalInput")
    byd = nc.dram_tensor("by", [2, P, NB], F32, kind="ExternalInput")
    biasd = nc.dram_tensor("bias", [1, 32], F16, kind="ExternalInput")
    outd = nc.dram_tensor("out", [UNITS, NPX, 64], F16, kind="ExternalOutput")
    dbgd = nc.dram_tensor("dbg", [32, 8 * NB], mybir.dt.int16,
                          kind="ExternalOutput")

    with tile.TileContext(nc) as tc, ExitStack() as ctx:
        cpool = ctx.enter_context(tc.tile_pool(name="const", bufs=1))
        ppool = ctx.enter_context(tc.tile_pool(name="psum", bufs=4, space="PSUM"))
        cvp = ctx.enter_context(tc.tile_pool(name="convtmp", bufs=4))
        fpool = ctx.enter_context(tc.tile_pool(name="fields", bufs=2))
        tpool = ctx.enter_context(tc.tile_pool(name="taps", bufs=1))
        opool = ctx.enter_context(tc.tile_pool(name="outt", bufs=2))
        mpool = ctx.enter_context(tc.tile_pool(name="mid", bufs=8))

        # ---- constants ----
        xc = []
        for k in range(2):
            t = cpool.tile([P, NPX], F16, tag=f"xc{k}")
            nc.sync.dma_start(t[:], x_cm[k])
            xc.append(t)
        wts = []
        for k in range(2):
            t = cpool.tile([P, 32], F16, tag=f"wt{k}")
            nc.sync.dma_start(t[:], wt[k])
            wts.append(t)
        bxs, bys = [], []
        for k in range(2):
            t = cpool.tile([P, NB], F32, tag=f"bx{k}")
            nc.sync.dma_start(t[:], bxd[k])
            bxs.append(t)
            t = cpool.tile([P, NB], F32, tag=f"by{k}")
            nc.sync.dma_start(t[:], byd[k])
            bys.append(t)
        biasb = cpool.tile([1, 32], F16, tag="biasb")
        nc.sync.dma_start(biasb[:], biasd[:])
        ones1 = cpool.tile([1, P], F16, tag="ones1")
        nc.vector.memset(ones1[:], 1.0)

        offT = cpool.tile([P, NB, 32], F32, tag="offT")

        # ---- conv (1x1) + bias + tanh ----
        for t in range(NB):
            ps = ppool.tile([P, 32], F32)
            nc.tensor.matmul(ps[:], xc[0][:, t * P:(t + 1) * P], wts[0][:],
                             start=True, stop=False)
            nc.tensor.matmul(ps[:], xc[1][:, t * P:(t + 1) * P], wts[1][:],
                             start=False, stop=False)
            nc.tensor.matmul(ps[:], ones1[:], biasb[:],
                             start=False, stop=True)
            nc.scalar.activation(offT[:, t, :], ps[:], AF.Tanh)

        # ---- per-unit fields, gather, combine ----
        for g in range(GROUPS):
            for i in range(2):
                for j in range(2):
                    u = g * 4 + i * 2 + j
                    ochx = 8 * g + 2 * i + j
                    ochy = ochx + 4

                    def fld(tag):
                        return fpool.tile([P, NB], F32, tag=tag, name=tag)

                    # gx = clamp(base_x + 6*tanh_off, 0, 95); x0 = floor(gx); fx frac
                    gxt = fld("gxt")
                    nc.vector.tensor_tensor(
                        gxt[:], offT[:, :, ochx], bxs[j][:], op=A.add)
                    gx = fld("gx")
                    nc.vector.scalar_tensor_tensor(
                        gx[:], offT[:, :, ochx], 5.0, gxt[:], A.mult, A.add)
                    nc.vector.tensor_scalar(gx[:], gx[:], 0.0, 95.0, A.max, A.min)
                    x0i = fpool.tile([P, NB], I32, tag="x0i")
                    nc.vector.tensor_copy(x0i[:], gx[:])
                    x0f = fld("x0f")
                    nc.vector.tensor_copy(x0f[:], x0i[:])
                    mx = fld("mx")
                    nc.vector.tensor_tensor(mx[:], x0f[:], gx[:], op=A.is_gt)
                    nc.vector.tensor_tensor(x0f[:], x0f[:], mx[:], op=A.subtract)
                    fx = fld("fx")
                    nc.vector.tensor_tensor(fx[:], gx[:], x0f[:], op=A.subtract)

                    gyt = fld("gyt")
                    nc.vector.tensor_tensor(
                        gyt[:], offT[:, :, ochy], bys[i][:], op=A.add)
                    gy = fld("gy")
                    nc.vector.scalar_tensor_tensor(
                        gy[:], offT[:, :, ochy], 5.0, gyt[:], A.mult, A.add)
                    nc.vector.tensor_scalar(gy[:], gy[:], 0.0, 95.0, A.max, A.min)
                    y0i = fpool.tile([P, NB], I32, tag="y0i")
                    nc.vector.tensor_copy(y0i[:], gy[:])
                    y0f = fld("y0f")
                    nc.vector.tensor_copy(y0f[:], y0i[:])
                    my = fld("my")
                    nc.vector.tensor_tensor(my[:], y0f[:], gy[:], op=A.is_gt)
                    nc.vector.tensor_tensor(y0f[:], y0f[:], my[:], op=A.subtract)
                    fy = fld("fy")
                    nc.vector.tensor_tensor(fy[:], gy[:], y0f[:], op=A.subtract)

                    idxf = fld("idxf")
                    nc.vector.scalar_tensor_tensor(
                        idxf[:], y0f[:], 96.0, x0f[:], A.mult, A.add)
                    idx16 = fpool.tile([P, NB], mybir.dt.int16, tag="idx16",
                                       name="idx16")
                    nc.vector.tensor_copy(idx16[:], idxf[:])
                    # build the [32, 576] wrapped idx table: idxq[q, 8b+r]
                    # = idx16[16r+q, b]; replicated on partitions 16-31
                    stag = fpool.tile([32, 8, NB], mybir.dt.int16, tag="stag",
                                      name="stag")
                    idxq = fpool.tile([32, 8 * NB], mybir.dt.int16, tag="idxq",
                                      name="idxq")
                    for r in range(8):
                        nc.sync.dma_start(stag[0:16, r, :],
                                          idx16[16 * r:16 * (r + 1), :])
                    for r in range(8):
                        nc.vector.tensor_copy(
                            idxq[0:16, r:8 * NB:8], stag[0:16, r, :])
                    nc.sync.dma_start(idxq[16:32, :], idxq[0:16, :])
                    if u == 0:
                        nc.sync.dma_start(dbgd[:], idxq[:])

                    fxb = fld("fxb")
                    nc.vector.tensor_scalar(fxb[:], fx[:], -1.0, 1.0, A.mult, A.add)
                    fyb = fld("fyb")
                    nc.vector.tensor_scalar(fyb[:], fy[:], -1.0, 1.0, A.mult, A.add)
                    wTL = fld("wTL")
                    nc.vector.tensor_tensor(wTL[:], fxb[:], fyb[:], op=A.mult)
                    wBL = fld("wBL")
                    nc.vector.tensor_tensor(wBL[:], fxb[:], fy[:], op=A.mult)
                    wTR = fld("wTR")
                    nc.vector.tensor_tensor(wTR[:], fx[:], fyb[:], op=A.mult)
                    wBR = fld("wBR")
                    nc.vector.tensor_tensor(wBR[:], fx[:], fy[:], op=A.mult)

                    out_u = outd[u].rearrange("(p b) c -> p b c", p=P)
                    for half in range(2):
                        T = tpool.tile([P, HB, 256], F16, tag="T")
                        in_ap = bass.AP(
                            xg.tensor if hasattr(xg, "tensor") else xg,
                            g * XG_ALLOC * 128,
                            [(128, XG_ROWS), (1, 256)])
                        for c in range(HB):
                            nc.gpsimd.dma_gather(
                                T[:, c, :].rearrange("p (a b) -> p a b", a=1),
                                in_ap,
                                idxq[:, half * 288 + c * 8:
                                     half * 288 + (c + 1) * 8],
                                num_idxs=128, num_idxs_reg=128,
                                elem_size=256, elem_step=128)
                        ot = opool.tile([P, HB, 64], F16, tag="ot")
                        for Bq in range(HB):
                            gb = half * HB + Bq
                            TL = T[:, Bq, 0:64]
                            BL = T[:, Bq, 64:128]
                            TR = T[:, Bq, 128:192]
                            BR = T[:, Bq, 192:256]
                            if True:
                                # DVE: tensor_scalar + scalar_tensor_tensor chain
                                a0 = mpool.tile([P, 64], F16, tag="a0v",
                                                name="a0v")
                                nc.vector.tensor_scalar(
                                    a0[:], TL, wTL[:, gb:gb + 1], None, A.mult)
                                a1 = mpool.tile([P, 64], F16, tag="a1v",
                                                name="a1v")
                                nc.vector.scalar_tensor_tensor(
                                    a1[:], BL, wBL[:, gb:gb + 1], a0[:],
                                    A.mult, A.add)
                                a2 = mpool.tile([P, 64], F16, tag="a2v",
                                                name="a2v")
                                nc.vector.scalar_tensor_tensor(
                                    a2[:], TR, wTR[:, gb:gb + 1], a1[:],
                                    A.mult, A.add)
                                nc.vector.scalar_tensor_tensor(
                                    ot[:, Bq, :], BR, wBR[:, gb:gb + 1], a2[:],
                                    A.mult, A.add)
                            else:
                                # GPSIMD: tensor_tensor-only chain
                                m0 = mpool.tile([P, 64], F16, tag="m0g",
                                                name="m0g")
                                nc.gpsimd.tensor_tensor(
                                    m0[:], TL,
                                    wTL[:, gb:gb + 1].to_broadcast([P, 64]),
                                    op=A.mult)
                                m1 = mpool.tile([P, 64], F16, tag="m1g",
                                                name="m1g")
                                nc.gpsimd.tensor_tensor(
                                    m1[:], BL,
                                    wBL[:, gb:gb + 1].to_broadcast([P, 64]),
                                    op=A.mult)
                                s0 = mpool.tile([P, 64], F16, tag="s0g",
                                                name="s0g")
                                nc.gpsimd.tensor_tensor(s0[:], m0[:], m1[:],
                                                        op=A.add)
                                m2 = mpool.tile([P, 64], F16, tag="m2g",
                                                name="m2g")
                                nc.gpsimd.tensor_tensor(
                                    m2[:], TR,
                                    wTR[:, gb:gb + 1].to_broadcast([P, 64]),
                                    op=A.mult)
                                m3 = mpool.tile([P, 64], F16, tag="m3g",
                                                name="m3g")
                                nc.gpsimd.tensor_tensor(
                                    m3[:], BR,
                                    wBR[:, gb:gb + 1].to_broadcast([P, 64]),
                                    op=A.mult)
                                s1 = mpool.tile([P, 64], F16, tag="s1g",
                                                name="s1g")
                                nc.gpsimd.tensor_tensor(s1[:], s0[:], m2[:],
                                                        op=A.add)
                                nc.gpsimd.tensor_tensor(ot[:, Bq, :], s1[:],
                                                        m3[:], op=A.add)
                        nc.sync.dma_start(
                            out_u[:, half * HB:(half + 1) * HB, :], ot[:])
    nc.finalize()
    return nc


def _prep_core(xb):
    """Host-side layout prep for one sample xb [256, 96, 96] fp32."""
    xflat = xb.reshape(256, NPX)
    x_cm = np.ascontiguousarray(xflat[:, _PERM]).astype(np.float16).reshape(2, P, NPX)
    Ag = xb.reshape(GROUPS, 64, 96, 96)
    D = np.zeros((GROUPS, XG_ALLOC, P), np.float16)
    Dv = D[:, :XG_ROWS].reshape(GROUPS, 97, 96, P)
    Dv[:, :96, :, 0:64] = Ag.transpose(0, 2, 3, 1)
    Dv[:, :95, :, 64:128] = Ag[:, :, 1:, :].transpose(0, 2, 3, 1)
    return x_cm, D.reshape(XG_TOT, P)


def _host_consts(w, b):
    wt = np.ascontiguousarray(w.T).astype(np.float16).reshape(2, P, 32)
    pix = (np.arange(P)[:, None] * NB + np.arange(NB)[None, :]).astype(np.float32)
    px_w = pix % 96
    px_h = pix // 96
    bx = np.stack([px_w - 0.25, px_w + 0.25]).astype(np.float32)
    by = np.stack([px_h - 0.25, px_h + 0.25]).astype(np.float32)
    bias = b.astype(np.float16).reshape(1, 32)
    return wt, bx, by, bias


def kernel(x, w, b):
    x = np.asarray(x, dtype=np.float32)
    w = np.asarray(w, dtype=np.float32)
    b = np.asarray(b, dtype=np.float32)
    Bn = x.shape[0]
    assert Bn == NCORES and x.shape[1:] == (256, 96, 96)

    if "nc" not in _CACHE:
        _CACHE["nc"] = _build_nc()
    nc = _CACHE["nc"]

    wt, bx, by, bias = _host_consts(w, b)
    in_maps = []
    for bi in range(Bn):
        x_cm, xgb = _prep_core(x[bi])
        in_maps.append({"x_cm": x_cm, "wt": wt, "xg": xgb,
                        "bx": bx, "by": by, "bias": bias})

    res = run_bass_kernel_spmd(nc, in_maps, list(range(NCORES)),
                               trace=bool(int(os.environ.get("KERNEL_TRACE", "0"))))
    kernel._last_results = res

    out = np.empty((Bn, 256, 192, 192), np.float32)
    for bi in range(Bn):
        o = res.results[bi]["out"].astype(np.float32)
        o = o.reshape(GROUPS, 2, 2, 96, 96, 64)
        out[bi] = o.transpose(0, 5, 3, 1, 4, 2).reshape(256, 192, 192)
    return out



# revision 2
# speedup vs baseline: 1.0011x; 1.0011x over previous
"""DySample (dynamic upsampling) Trainium2 Bass kernel.

Math (per sample):
  off = tanh(pixel_shuffle(w @ x + b))            # per-pixel offsets
  grid = static_base + off * 0.125                # normalized coords
  out  = bilinear_border_sample(x_grouped, grid)  # [4g, 64c, 192, 192]

Strategy: pure data-parallel over batch B=8 -> 1 sample per NeuronCore.
v2 changes vs v1 (which spent 90% of the kernel in SWDGE fixed overhead,
1152 gather calls x ~1.9us):
  - 64 dma_gather calls (num_idxs=2304, single_packet=False to dodge the
    64-descriptor-per-packet ceiling) instead of 1152 calls of 128 —
    amortizes the ~1us SWDGE per-instruction fixed cost.
  - Field computation batched across all 16 units per image half
    ([P, 16, 36] ops) instead of per-unit [P, 72] ops.
  - Tap combine uses free-dim-broadcast weight APs: 7 big DVE ops of
    [P, 18, 64] per gather tile instead of 4 ops per 128-pixel block.
Per core: PE does the 1x1 conv into PSUM; ACT applies tanh; DVE builds
gather indices + bilinear tap weights; SWDGE gathers 512B quads
[TL|BL|TR|BR] from a host-staged row-pair-interleaved image; DVE applies
the 4 tap weights; HWDGE stores fp16 outputs. Host reassembles (pure
layout + dtype cast).
"""

import os
from contextlib import ExitStack

import numpy as np

import concourse.bass as bass
import concourse.tile as tile
from concourse import bacc
from concourse import mybir
from concourse.bass_utils import run_bass_kernel_spmd

F16 = mybir.dt.float16
F32 = mybir.dt.float32
I32 = mybir.dt.int32
I16 = mybir.dt.int16
A = mybir.AluOpType
AF = mybir.ActivationFunctionType

P = 128          # partitions
NPX = 9216       # 96*96 conv-resolution pixels per (group, parity) unit
NB = NPX // P    # 72 free-dim blocks per unit
HB = NB // 2     # 36 blocks per half
QB = HB // 2     # 18 blocks per gather call
GROUPS = 4
UNITS = 16       # (g, i, j)
XG_ROWS = 97 * 96           # rows per group in the interleaved gather image
XG_ALLOC = XG_ROWS + 2      # +2 pad rows so the overlapping gather AP stays in bounds
XG_TOT = GROUPS * XG_ALLOC
NCORES = 8
NIDX = P * QB    # 2304 indices per gather call

# column j of the channel-major conv input holds flat pixel (j%128)*72 + j//128,
# so conv tile t / psum partition p <-> pixel p*72 + t  (partition-major raster)
_PERM = (np.arange(NPX) % P) * NB + (np.arange(NPX) // P)

# output-channel permutation: och_new[u] = x-offset chan of unit u,
# och_new[16+u] = y-offset chan (u = g*4 + i*2 + j)
_U = np.arange(UNITS)
_OCH_X = 8 * (_U // 4) + 2 * ((_U // 2) % 2) + (_U % 2)
_OCH_PERM = np.concatenate([_OCH_X, _OCH_X + 4])

_CACHE = {}


def _build_nc():
    nc = bacc.Bacc("TRN2", target_bir_lowering=False,
                   dynamic_dma_scratch_size=65536)
    x_cm = nc.dram_tensor("x_cm", [2, P, NPX], F16, kind="ExternalInput")
    wt = nc.dram_tensor("wt", [2, P, 32], F16, kind="ExternalInput")
    xg = nc.dram_tensor("xg", [XG_TOT, P], F16, kind="ExternalInput")
    bxd = nc.dram_tensor("bx", [P, UNITS, NB], F32, kind="ExternalInput")
    byd = nc.dram_tensor("by", [P, UNITS, NB], F32, kind="ExternalInput")
    biasd = nc.dram_tensor("bias", [1, 32], F16, kind="ExternalInput")
    outd = nc.dram_tensor("out", [UNITS, NPX, 64], F16, kind="ExternalOutput")

    with tile.TileContext(nc) as tc, ExitStack() as ctx:
        cpool = ctx.enter_context(tc.tile_pool(name="const", bufs=1))
        ppool = ctx.enter_context(tc.tile_pool(name="psum", bufs=4, space="PSUM"))
        fpool = ctx.enter_context(tc.tile_pool(name="fields", bufs=1))
        wpool = ctx.enter_context(tc.tile_pool(name="wq", bufs=2))
        spool = ctx.enter_context(tc.tile_pool(name="stag", bufs=1))
        qpool = ctx.enter_context(tc.tile_pool(name="idxq", bufs=2))
        tpool = ctx.enter_context(tc.tile_pool(name="taps", bufs=3))
        opool = ctx.enter_context(tc.tile_pool(name="outt", bufs=3))
        mpool = ctx.enter_context(tc.tile_pool(name="mid", bufs=2))

        # ---- constants ----
        xc = []
        for k in range(2):
            t = cpool.tile([P, NPX], F16, tag=f"xc{k}")
            nc.sync.dma_start(t[:], x_cm[k])
            xc.append(t)
        wts = []
        for k in range(2):
            t = cpool.tile([P, 32], F16, tag=f"wt{k}")
            nc.sync.dma_start(t[:], wt[k])
            wts.append(t)
        bxq = cpool.tile([P, UNITS, NB], F32, tag="bxq")
        nc.sync.dma_start(bxq[:], bxd[:])
        byq = cpool.tile([P, UNITS, NB], F32, tag="byq")
        nc.sync.dma_start(byq[:], byd[:])
        biasb = cpool.tile([1, 32], F16, tag="biasb")
        nc.sync.dma_start(biasb[:], biasd[:])
        ones1 = cpool.tile([1, P], F16, tag="ones1")
        nc.vector.memset(ones1[:], 1.0)

        # tanh(conv) output, och-major: offU[:, c, t] for psum partition p,
        # och c, block t  <-> pixel p*72 + t
        offU = cpool.tile([P, 32, NB], F32, tag="offU")

        for h in range(2):
            t0, t1 = h * HB, (h + 1) * HB
            # ---- conv (1x1) + bias + tanh for this half ----
            for t in range(t0, t1):
                ps = ppool.tile([P, 32], F32)
                nc.tensor.matmul(ps[:], xc[0][:, t * P:(t + 1) * P], wts[0][:],
                                 start=True, stop=False)
                nc.tensor.matmul(ps[:], xc[1][:, t * P:(t + 1) * P], wts[1][:],
                                 start=False, stop=False)
                nc.tensor.matmul(ps[:], ones1[:], biasb[:],
                                 start=False, stop=True)
                nc.scalar.activation(offU[:, :, t], ps[:], AF.Tanh)

            # ---- fields for this half, batched over all 16 units ----
            offx = offU[:, 0:UNITS, t0:t1]
            offy = offU[:, UNITS:2 * UNITS, t0:t1]
            bxs = bxq[:, :, t0:t1]
            bys = byq[:, :, t0:t1]

            def fld(tag, dt=F32):
                return fpool.tile([P, UNITS, HB], dt, tag=tag, name=tag)

            def wfld(tag, dt=F16):
                return wpool.tile([P, UNITS, HB], dt, tag=tag, name=tag)

            # gx = clamp(base_x + 6*tanh_off, 0, 95); x0 = floor(gx); fx frac
            gx = fld("g")
            nc.vector.scalar_tensor_tensor(gx[:], offx, 6.0, bxs, A.mult, A.add)
            nc.vector.tensor_scalar(gx[:], gx[:], 0.0, 95.0, A.max, A.min)
            x0i = fld("ti", I32)
            nc.vector.tensor_copy(x0i[:], gx[:])
            x0f = fld("x0f")
            nc.vector.tensor_copy(x0f[:], x0i[:])
            mx = fld("m")
            nc.vector.tensor_tensor(mx[:], x0f[:], gx[:], op=A.is_gt)
            nc.vector.tensor_tensor(x0f[:], x0f[:], mx[:], op=A.subtract)
            fx = fld("fx")
            nc.vector.tensor_tensor(fx[:], gx[:], x0f[:], op=A.subtract)

            gy = fld("g")
            nc.vector.scalar_tensor_tensor(gy[:], offy, 6.0, bys, A.mult, A.add)
            nc.vector.tensor_scalar(gy[:], gy[:], 0.0, 95.0, A.max, A.min)
            y0i = fld("ti", I32)
            nc.vector.tensor_copy(y0i[:], gy[:])
            y0f = fld("y0f")
            nc.vector.tensor_copy(y0f[:], y0i[:])
            my = fld("m")
            nc.vector.tensor_tensor(my[:], y0f[:], gy[:], op=A.is_gt)
            nc.vector.tensor_tensor(y0f[:], y0f[:], my[:], op=A.subtract)
            fy = fld("fy")
            nc.vector.tensor_tensor(fy[:], gy[:], y0f[:], op=A.subtract)

            idxf = fld("g")
            nc.vector.scalar_tensor_tensor(
                idxf[:], y0f[:], 96.0, x0f[:], A.mult, A.add)
            idx16 = wfld("idx16", I16)
            nc.vector.tensor_copy(idx16[:], idxf[:])

            fxb = fld("fxb")
            nc.vector.tensor_scalar(fxb[:], fx[:], -1.0, 1.0, A.mult, A.add)
            fyb = fld("x0f")
            nc.vector.tensor_scalar(fyb[:], fy[:], -1.0, 1.0, A.mult, A.add)
            wTL = wfld("wTL")
            nc.vector.tensor_tensor(wTL[:], fxb[:], fyb[:], op=A.mult)
            wBL = wfld("wBL")
            nc.vector.tensor_tensor(wBL[:], fxb[:], fy[:], op=A.mult)
            wTR = wfld("wTR")
            nc.vector.tensor_tensor(wTR[:], fx[:], fyb[:], op=A.mult)
            wBR = wfld("wBR")
            nc.vector.tensor_tensor(wBR[:], fx[:], fy[:], op=A.mult)

            # ---- wrapped gather-index tables for all 16 units ----
            # idxq[q, u, 8*B + r] = idx16[16r+q, u, B]
            stag = spool.tile([16, UNITS, 8, HB], I16, tag="stag", name="stag")
            idxq = qpool.tile([32, UNITS, 8 * HB], I16, tag="idxq", name="idxq")
            for r in range(8):
                nc.sync.dma_start(stag[0:16, :, r, :],
                                  idx16[16 * r:16 * (r + 1), :, :])
            for r in range(8):
                nc.vector.tensor_copy(
                    idxq[0:16, :, r:8 * HB:8], stag[0:16, :, r, :])
            nc.sync.dma_start(idxq[16:32, :, :], idxq[0:16, :, :])

            # ---- per-unit-quarter gather + combine ----
            for u in range(UNITS):
                g = u // 4
                in_ap = bass.AP(
                    xg.tensor if hasattr(xg, "tensor") else xg,
                    g * XG_ALLOC * 128,
                    [(128, XG_ROWS), (1, 256)])
                out_u = outd[u].rearrange("(p b) c -> p b c", p=P)
                for q in range(2):
                    T = tpool.tile([P, QB, 256], F16, tag="T")
                    nc.gpsimd.dma_gather(
                        T[:], in_ap,
                        idxq[:, u, q * (8 * QB):(q + 1) * (8 * QB)],
                        num_idxs=NIDX, num_idxs_reg=NIDX,
                        elem_size=256, elem_step=128,
                        single_packet=False)

                    b0, b1 = q * QB, (q + 1) * QB

                    def wb(W):
                        return W[:, u, b0:b1].to_broadcast([P, QB, 64])

                    ot = opool.tile([P, QB, 64], F16, tag="ot")
                    ma = mpool.tile([P, QB, 64], F16, tag="ma", name="ma")
                    nc.vector.tensor_tensor(ma[:], T[:, :, 0:64], wb(wTL),
                                            op=A.mult)
                    mt = mpool.tile([P, QB, 64], F16, tag="mt", name="mt")
                    nc.vector.tensor_tensor(mt[:], T[:, :, 64:128], wb(wBL),
                                            op=A.mult)
                    mb = mpool.tile([P, QB, 64], F16, tag="mb", name="mb")
                    nc.vector.tensor_tensor(mb[:], ma[:], mt[:], op=A.add)
                    mt2 = mpool.tile([P, QB, 64], F16, tag="mt", name="mt2")
                    nc.vector.tensor_tensor(mt2[:], T[:, :, 128:192], wb(wTR),
                                            op=A.mult)
                    ma2 = mpool.tile([P, QB, 64], F16, tag="ma", name="ma2")
                    nc.vector.tensor_tensor(ma2[:], mb[:], mt2[:], op=A.add)
                    mt3 = mpool.tile([P, QB, 64], F16, tag="mt", name="mt3")
                    nc.vector.tensor_tensor(mt3[:], T[:, :, 192:256], wb(wBR),
                                            op=A.mult)
                    nc.vector.tensor_tensor(ot[:], ma2[:], mt3[:], op=A.add)

                    nc.sync.dma_start(
                        out_u[:, t0 + b0:t0 + b1, :], ot[:])
    nc.finalize()
    return nc


def _prep_core(xb):
    """Host-side layout prep for one sample xb [256, 96, 96] fp32."""
    xflat = xb.reshape(256, NPX)
    x_cm = np.ascontiguousarray(xflat[:, _PERM]).astype(np.float16).reshape(2, P, NPX)
    Ag = xb.reshape(GROUPS, 64, 96, 96)
    D = np.zeros((GROUPS, XG_ALLOC, P), np.float16)
    Dv = D[:, :XG_ROWS].reshape(GROUPS, 97, 96, P)
    Dv[:, :96, :, 0:64] = Ag.transpose(0, 2, 3, 1)
    Dv[:, :95, :, 64:128] = Ag[:, :, 1:, :].transpose(0, 2, 3, 1)
    return x_cm, D.reshape(XG_TOT, P)


def _host_consts(w, b):
    wp = w[_OCH_PERM]                        # [32, 256] reordered och
    wt = np.ascontiguousarray(wp.T).astype(np.float16).reshape(2, P, 32)
    bias = b[_OCH_PERM].astype(np.float16).reshape(1, 32)
    pix = (np.arange(P)[:, None] * NB + np.arange(NB)[None, :]).astype(np.float32)
    px_w = pix % 96
    px_h = pix // 96
    # bx[p, u, B] = px_w +- 0.25 per unit's j; by per unit's i
    j_u = (_U % 2).astype(np.float32) * 0.5 - 0.25
    i_u = ((_U // 2) % 2).astype(np.float32) * 0.5 - 0.25
    bx = (px_w[:, None, :] + j_u[None, :, None]).astype(np.float32)
    by = (px_h[:, None, :] + i_u[None, :, None]).astype(np.float32)
    return wt, bx, by, bias


def kernel(x, w, b):
    x = np.asarray(x, dtype=np.float32)
    w = np.asarray(w, dtype=np.float32)
    b = np.asarray(b, dtype=np.float32)
    Bn = x.shape[0]
    assert Bn == NCORES and x.shape[1:] == (256, 96, 96)

    if "nc" not in _CACHE:
        _CACHE["nc"] = _build_nc()
    nc = _CACHE["nc"]

    wt, bx, by, bias = _host_consts(w, b)
    in_maps = []
    for bi in range(Bn):
        x_cm, xgb = _prep_core(x[bi])
        in_maps.append({"x_cm": x_cm, "wt": wt, "xg": xgb,
                        "bx": bx, "by": by, "bias": bias})

    res = run_bass_kernel_spmd(nc, in_maps, list(range(NCORES)),
                               trace=bool(int(os.environ.get("KERNEL_TRACE", "0"))))
    kernel._last_results = res

    out = np.empty((Bn, 256, 192, 192), np.float32)
    for bi in range(Bn):
        o = res.results[bi]["out"].astype(np.float32)
        o = o.reshape(GROUPS, 2, 2, 96, 96, 64)
        out[bi] = o.transpose(0, 5, 3, 1, 4, 2).reshape(256, 192, 192)
    return out
